# revision 1
# baseline (speedup 1.0000x reference)
"""Trainium2 Bass kernel for nn_AdaptiveSampler (sparse grid_sample attention).

Strategy (data-parallel over batch, 8 cores x 4 batch items each):
  - Host: features reshaped channels-last [B*H*W, C] in bf16 so every
    spatial cell is one contiguous 2KB row -> indirect row gathers.
  - Device per core:
      phase A: keypoint -> bilinear corner cells/weights (DVE f32 math)
      seed    = dma_gather(4 corners x 512 keypoints) -> weighted reduce
      MLPs    = PE matmuls (offsets + attention logits), softmax on DVE/ACT
      phase B: per keypoint a 4x4 patch around the seed cell covers all
               16 sample corners; per-cell weights are built by position
               selects (d = floor(px) - patch_base), folding attention
               softmax + bilinear + border validity into one weight.
      fused   = dma_gather of 8KB patch rows (4 x-cells, 4 rows/keypoint)
                * broadcast weights, segment-reduce, PE-transpose, DMA out.
All computation (gathers, MLPs, softmax, bilinear) happens on-device; the
host only reorders input layout and concatenates per-core outputs.
"""

import os
import sys
from contextlib import ExitStack

import numpy as np

sys.path.insert(0, "/opt/trn_rl_repo")

import ml_dtypes

import concourse.bass as bass
import concourse.tile as tile
from concourse import bacc, mybir

F32 = mybir.dt.float32
BF16 = mybir.dt.bfloat16
I16 = mybir.dt.int16

ALU = mybir.AluOpType
ACT = mybir.ActivationFunctionType
AX = mybir.AxisListType

B = 4          # batch items per core
C = 1024       # channels
H = W = 64
HW = H * W     # 4096 cells per batch item
J = 128        # keypoints
NP = 4         # sample points per keypoint
Q = C // 128   # 8 channel chunks
NIDX = J * 16  # 2048 indices per gather set (seed corners / main patch rows)
TWO23 = float(2 ** 23)


def _floor(nc, pool, src, shape, tag):
    """floor(src) on DVE via round-to-nearest + correction. Returns tile."""
    rnd = pool.tile(list(shape), F32, tag=f"floor_rnd_{tag}")
    nc.vector.tensor_scalar(rnd[:], src, TWO23, TWO23, ALU.add, ALU.subtract)
    flo = pool.tile(list(shape), F32, tag=f"floor_out_{tag}")
    nc.vector.tensor_tensor(flo[:], src, rnd[:], ALU.is_lt)
    nc.vector.tensor_tensor(flo[:], rnd[:], flo[:], ALU.subtract)
    return flo


def build_nc():
    nc = bacc.Bacc()

    feat = nc.declare_dram_parameter("feat", [B * HW, C], BF16, isOutput=False)
    kp = nc.declare_dram_parameter("kp", [J, 2 * B], F32, isOutput=False)
    w1o = nc.declare_dram_parameter("w1o", [128, Q, 128], BF16, isOutput=False)
    w1a = nc.declare_dram_parameter("w1a", [128, Q, 128], BF16, isOutput=False)
    w2o = nc.declare_dram_parameter("w2o", [128, 8], BF16, isOutput=False)
    w2a = nc.declare_dram_parameter("w2a", [128, 4], BF16, isOutput=False)
    b1o = nc.declare_dram_parameter("b1o", [128, 1], F32, isOutput=False)
    b1a = nc.declare_dram_parameter("b1a", [128, 1], F32, isOutput=False)
    b2o = nc.declare_dram_parameter("b2o", [8, 1], F32, isOutput=False)
    b2a = nc.declare_dram_parameter("b2a", [4, 1], F32, isOutput=False)
    bbase = nc.declare_dram_parameter("bbase", [128, B], F32, isOutput=False)
    posc = nc.declare_dram_parameter("posc", [128, 4], F32, isOutput=False)
    ident = nc.declare_dram_parameter("ident", [128, 128], F32, isOutput=False)
    identb = nc.declare_dram_parameter("identb", [128, 128], BF16, isOutput=False)
    out = nc.declare_dram_parameter("out", [B * J, C], BF16, isOutput=True)

    # DRAM scratch for flattening per-column weights before partition bcast
    wscr_s = nc.dram_tensor("wscr_s", [J * 16], BF16)
    wscr_m = nc.dram_tensor("wscr_m", [J * 48], BF16)

    # Overlapping row view of feat: row i = cells i..i+3 (8KB), for patch
    # gathers. Max row start 16380 -> read end == tensor end exactly.
    feat_ov = bass.AP(feat[:].tensor, 0, [[C, B * HW - 2], [1, 3 * C]])

    with ExitStack() as ctx:
        tc = ctx.enter_context(tile.TileContext(nc))
        cons = ctx.enter_context(tc.tile_pool(name="cons", bufs=1))
        a = ctx.enter_context(tc.tile_pool(name="phaseA", bufs=1))
        gp = ctx.enter_context(tc.tile_pool(name="gather", bufs=3))
        mgp = ctx.enter_context(tc.tile_pool(name="mgpool", bufs=12))
        wp = ctx.enter_context(tc.tile_pool(name="wbc", bufs=1))
        op = ctx.enter_context(tc.tile_pool(name="outT", bufs=2))
        ip = ctx.enter_context(tc.tile_pool(name="idxw", bufs=2))
        ps = ctx.enter_context(tc.tile_pool(name="psT", bufs=3, space="PSUM"))
        pmm = ctx.enter_context(tc.tile_pool(name="psMM", bufs=2, space="PSUM"))

        # ---------------- constants ----------------
        def c_load(name, shape, dt, src):
            t = cons.tile(shape, dt, tag=name)
            nc.sync.dma_start(out=t[:], in_=src)
            return t

        kp_sb = c_load("kp", [J, B, 2], F32, kp[:].rearrange("j (b t) -> j b t", t=2))
        w1o_sb = c_load("w1o", [128, Q, 128], BF16, w1o[:])
        w1a_sb = c_load("w1a", [128, Q, 128], BF16, w1a[:])
        w2o_sb = c_load("w2o", [128, 8], BF16, w2o[:])
        w2a_sb = c_load("w2a", [128, 4], BF16, w2a[:])
        b1o_sb = c_load("b1o", [128, 1], F32, b1o[:])
        b1a_sb = c_load("b1a", [128, 1], F32, b1a[:])
        b2o_sb = c_load("b2o", [8, 1], F32, b2o[:])
        b2a_sb = c_load("b2a", [4, 1], F32, b2a[:])
        bbase_sb = c_load("bbase", [128, B], F32, bbase[:])
        posc_sb = c_load("posc", [128, 4], F32, posc[:])
        id_sb = c_load("ident", [128, 128], F32, ident[:])
        idb_sb = c_load("identb", [128, 128], BF16, identb[:])

        # ---------------- phase A: seed corners ----------------
        ix = a.tile([J, B], F32)
        nc.vector.tensor_scalar(ix[:], kp_sb[:, :, 0], 31.5, 31.5, ALU.mult, ALU.add)
        iy = a.tile([J, B], F32)
        nc.vector.tensor_scalar(iy[:], kp_sb[:, :, 1], 31.5, 31.5, ALU.mult, ALU.add)

        x0 = _floor(nc, a, ix[:], (J, B), "x0")
        y0 = _floor(nc, a, iy[:], (J, B), "y0")

        def pair_and_weights(base, i_coord, tagp):
            p = a.tile([J, B, 2], F32, tag=f"{tagp}_p")
            wgt = a.tile([J, B, 2], F32, tag=f"{tagp}_w")
            nc.vector.tensor_copy(p[:, :, 0], base[:])
            nc.vector.tensor_scalar_add(p[:, :, 1], base[:], 1.0)
            nc.vector.tensor_tensor(wgt[:, :, 1], i_coord, base[:], ALU.subtract)
            nc.vector.tensor_scalar(
                wgt[:, :, 0], wgt[:, :, 1], -1.0, 1.0, ALU.mult, ALU.add
            )
            return p, wgt

        xp, wxp = pair_and_weights(x0, ix[:], "x")
        yp, wyp = pair_and_weights(y0, iy[:], "y")

        # seed cell idx [J, B, 2cy, 2cx] = bbase + yp*64 + xp
        idx4 = a.tile([J, B, 2, 2], F32)
        t1 = a.tile([J, B, 2], F32)
        nc.vector.tensor_scalar_mul(t1[:], yp[:], 64.0)
        nc.vector.tensor_tensor(
            idx4[:],
            t1[:].unsqueeze(3).to_broadcast((J, B, 2, 2)),
            xp[:].unsqueeze(2).to_broadcast((J, B, 2, 2)),
            ALU.add,
        )
        nc.vector.tensor_tensor(
            idx4[:],
            idx4[:],
            bbase_sb[:].unsqueeze(2).unsqueeze(3).to_broadcast((J, B, 2, 2)),
            ALU.add,
        )
        w4 = a.tile([J, B, 2, 2], F32)
        nc.vector.tensor_tensor(
            w4[:],
            wyp[:].unsqueeze(3).to_broadcast((J, B, 2, 2)),
            wxp[:].unsqueeze(2).to_broadcast((J, B, 2, 2)),
            ALU.mult,
        )

        def wrap_idx(idx_flat_ap):
            """[J,16] f32 cell ids -> wrapped+replicated [128, J] int16 tile."""
            rep = ip.tile([J, 8, 16], F32, tag="idxrep")
            for g in range(8):
                nc.vector.tensor_copy(rep[:, g, :], idx_flat_ap)
            psT = ps.tile([128, J], F32, tag="tp")
            nc.tensor.transpose(
                psT[:], rep[:].rearrange("j g c -> j (g c)"), id_sb[:, :J]
            )
            idxw = ip.tile([128, J], I16, tag="idxw")
            nc.vector.tensor_copy(idxw[:], psT[:])
            return idxw

        def bcast_weights(w_flat_ap, wscr, n, slot, dest_view=None,
                          split_x=None):
            """[J, n] f32 col-weights -> [128, J*n] bf16 via DRAM bounce.
            dest_view(wscr_ap) may reorder the DRAM layout. split_x: read
            back in two j-half DMAs over an (x, j, c) DRAM layout."""
            wb16 = a.tile([J, n], BF16, tag=f"wb16_{slot}")
            nc.vector.tensor_copy(wb16[:], w_flat_ap)
            dst = (
                dest_view(wscr[:])
                if dest_view is not None
                else wscr[:].rearrange("(j c) -> j c", c=n)
            )
            nc.sync.dma_start(out=dst, in_=wb16[:])
            wbc = wp.tile([128, J * n], BF16, tag=f"wbc_{slot}")
            if split_x is None:
                nc.sync.dma_start(
                    out=wbc[:],
                    in_=wscr[:].unsqueeze(0).to_broadcast((128, J * n)),
                )
            else:
                xs, cs = split_x, (J * n) // split_x // 2
                wv = wbc[:].rearrange("p (x jc) -> p x jc", x=xs)
                sv = (
                    wscr[:]
                    .rearrange("(x jc) -> x jc", x=xs)
                    .unsqueeze(0)
                    .to_broadcast((128, xs, 2 * cs))
                )
                for jh in range(2):
                    nc.sync.dma_start(
                        out=wv[:, :, jh * cs : (jh + 1) * cs],
                        in_=sv[:, :, jh * cs : (jh + 1) * cs],
                    )
            return wbc

        idxw_seed = wrap_idx(idx4[:].rearrange("j b cy cx -> j (b cy cx)"))
        wbc_seed = bcast_weights(
            w4[:].rearrange("j b cy cx -> j (b cy cx)"), wscr_s, 16, "s"
        )

        # ---------------- phase B: 4x4 patch per keypoint ----------------
        # patch base bx/by [J, B] = clip(seed_corner - 1, 0, 60)
        rx = a.tile([J, B], F32)
        nc.vector.tensor_scalar(rx[:], ix[:], TWO23, TWO23, ALU.add, ALU.subtract)
        bx = a.tile([J, B], F32)
        nc.vector.tensor_scalar(bx[:], rx[:], -1.0, 0.0, ALU.add, ALU.max)
        nc.vector.tensor_scalar_min(bx[:], bx[:], 61.0)
        by = a.tile([J, B], F32)
        nc.vector.tensor_scalar(by[:], y0[:], -1.0, 0.0, ALU.add, ALU.max)
        nc.vector.tensor_scalar_min(by[:], by[:], 60.0)

        # patch row ids [J, B, 4Y] = bbase + (by + Y)*64 + bx
        pbase = a.tile([J, B], F32)
        nc.vector.tensor_scalar_mul(pbase[:], by[:], 64.0)
        nc.vector.tensor_tensor(pbase[:], pbase[:], bx[:], ALU.add)
        nc.vector.tensor_tensor(pbase[:], pbase[:], bbase_sb[:], ALU.add)
        y64 = a.tile([128, 4], F32)
        nc.vector.tensor_scalar_mul(y64[:], posc_sb[:], 64.0)
        idxp = a.tile([J, B, 4], F32)
        nc.vector.tensor_tensor(
            idxp[:],
            pbase[:].unsqueeze(2).to_broadcast((J, B, 4)),
            y64[:].unsqueeze(1).to_broadcast((J, B, 4)),
            ALU.add,
        )

        idxw_m = wrap_idx(idxp[:].rearrange("j b y -> j (b y)"))

        # ---------------- seed gather + combine ----------------
        HN = 256  # seed chunk: 256 idx x 2KB rows (130 descs -> pipelined)
        seed = a.tile([128, Q, J * B], BF16)
        for h in range(8):
            seedg = gp.tile([128, Q, HN], BF16, tag="seedg")
            nc.gpsimd.dma_gather(
                seedg[:],
                feat[:],
                idxw_seed[:, 16 * h : 16 * h + 16],
                num_idxs=HN,
                num_idxs_reg=HN,
                elem_size=C,
                transpose=True,
            )
            with nc.allow_low_precision("bf16 grid-sample compute"):
                nc.vector.tensor_tensor(
                    seedg[:],
                    seedg[:],
                    wbc_seed[:, HN * h : HN * (h + 1)]
                    .unsqueeze(1)
                    .to_broadcast((128, Q, HN)),
                    ALU.mult,
                )
                nc.vector.tensor_reduce(
                    seed[:, :, 64 * h : 64 * (h + 1)],
                    seedg[:].rearrange("p q (jb c) -> p (q jb) c", c=4),
                    AX.X,
                    ALU.add,
                )

        # ---------------- main patch gathers (independent of MLPs) --------
        MN = 128  # idxs per chunk: 8 keypoints x (4b x 4Y) 6KB patch rows
        mgs = []
        for h in range(16):
            mg = mgp.tile([128, 3 * Q, MN], BF16, tag="mg")
            nc.gpsimd.dma_gather(
                mg[:],
                feat_ov,
                idxw_m[:, 8 * h : 8 * h + 8],
                num_idxs=MN,
                num_idxs_reg=MN,
                elem_size=3 * C,
                elem_step=C,
                transpose=True,
            )
            mgs.append(mg)

        # ---------------- MLPs ----------------
        def mlp_head(w1_sb, b1_sb, name):
            hps = pmm.tile([128, J * B], F32, tag="mm")
            for q in range(Q):
                nc.tensor.matmul(
                    hps[:],
                    w1_sb[:, q, :],
                    seed[:, q, :],
                    start=(q == 0),
                    stop=(q == Q - 1),
                )
            h_sb = a.tile([128, J * B], BF16, tag=f"hsb_{name}")
            nc.scalar.activation(h_sb[:], hps[:], ACT.Relu, bias=b1_sb[:])
            return h_sb

        h_off = mlp_head(w1o_sb, b1o_sb, "off")
        h_att = mlp_head(w1a_sb, b1a_sb, "att")

        ops2 = pmm.tile([8, J * B], F32, tag="mm")
        nc.tensor.matmul(ops2[:], w2o_sb[:], h_off[:], start=True, stop=True)
        off2 = a.tile([8, J * B], F32)
        nc.scalar.activation(off2[:], ops2[:], ACT.Identity, bias=b2o_sb[:])

        aps2 = pmm.tile([4, J * B], F32, tag="mm")
        nc.tensor.matmul(aps2[:], w2a_sb[:], h_att[:], start=True, stop=True)
        att2 = a.tile([4, J * B], F32)
        nc.scalar.activation(att2[:], aps2[:], ACT.Identity, bias=b2a_sb[:])

        # transpose MLP outputs back to [J, B, ch] layout (per-b strided cols)
        offT = a.tile([J, B, 8], F32)
        attT = a.tile([J, B, 4], F32)
        for b in range(B):
            pso = ps.tile([J, 8], F32, tag="tp")
            nc.tensor.transpose(pso[:], off2[:, b::B], id_sb[:8, :8])
            nc.scalar.copy(offT[:, b, :], pso[:])
            psa = ps.tile([J, 4], F32, tag="tp")
            nc.tensor.transpose(psa[:], att2[:, b::B], id_sb[:4, :4])
            nc.scalar.copy(attT[:, b, :], psa[:])

        # per-point coords px/py [J, B, NP]
        px = a.tile([J, B, NP], F32)
        nc.vector.tensor_tensor(
            px[:],
            ix[:].unsqueeze(2).to_broadcast((J, B, NP)),
            offT[:, :, 0:NP],
            ALU.add,
        )
        py = a.tile([J, B, NP], F32)
        nc.vector.tensor_tensor(
            py[:],
            iy[:].unsqueeze(2).to_broadcast((J, B, NP)),
            offT[:, :, NP : 2 * NP],
            ALU.add,
        )

        # softmax over NP  [J, B, NP]
        amax = a.tile([J, B, 1], F32)
        nc.vector.tensor_reduce(amax[:], attT[:], AX.X, ALU.max)
        ae = a.tile([J, B, NP], F32)
        nc.vector.tensor_tensor(
            ae[:], attT[:], amax[:].to_broadcast((J, B, NP)), ALU.subtract
        )
        nc.scalar.activation(ae[:], ae[:], ACT.Exp)
        asum = a.tile([J, B, 1], F32)
        nc.vector.tensor_reduce(asum[:], ae[:], AX.X, ALU.add)
        nc.vector.reciprocal(asum[:], asum[:])
        attw = a.tile([J, B, NP], F32)
        nc.vector.tensor_tensor(
            attw[:], ae[:], asum[:].to_broadcast((J, B, NP)), ALU.mult
        )

        def axis_select(pc, base, tagp, npos=4):
            """Position-select weights [J, B, NP, npos]:
            w0*(pos==d) + w1*(pos==d+1), d = floor(pc) - base."""
            c0 = _floor(nc, a, pc[:], (J, B, NP), tagp)
            w1t = a.tile([J, B, NP], F32, tag=f"{tagp}_w1")
            nc.vector.tensor_tensor(w1t[:], pc[:], c0[:], ALU.subtract)
            w0t = a.tile([J, B, NP], F32, tag=f"{tagp}_w0")
            nc.vector.tensor_scalar(w0t[:], w1t[:], -1.0, 1.0, ALU.mult, ALU.add)
            d = a.tile([J, B, NP], F32, tag=f"{tagp}_d")
            nc.vector.tensor_tensor(
                d[:], c0[:], base[:].unsqueeze(2).to_broadcast((J, B, NP)),
                ALU.subtract,
            )
            d1 = a.tile([J, B, NP], F32, tag=f"{tagp}_d1")
            nc.vector.tensor_scalar_add(d1[:], d[:], 1.0)
            posb = (
                posc_sb[:, 0:npos]
                .unsqueeze(1)
                .unsqueeze(2)
                .to_broadcast((J, B, NP, npos))
            )
            sel = a.tile([J, B, NP, npos], F32, tag=f"{tagp}_sel")
            eq = a.tile([J, B, NP, npos], F32, tag=f"{tagp}_eq")
            nc.vector.tensor_tensor(
                eq[:], d[:].unsqueeze(3).to_broadcast((J, B, NP, npos)), posb,
                ALU.is_equal,
            )
            nc.vector.tensor_tensor(
                sel[:], eq[:], w0t[:].unsqueeze(3).to_broadcast((J, B, NP, npos)),
                ALU.mult,
            )
            nc.vector.tensor_tensor(
                eq[:], d1[:].unsqueeze(3).to_broadcast((J, B, NP, npos)), posb,
                ALU.is_equal,
            )
            nc.vector.tensor_tensor(
                eq[:], eq[:], w1t[:].unsqueeze(3).to_broadcast((J, B, NP, npos)),
                ALU.mult,
            )
            nc.vector.tensor_tensor(sel[:], sel[:], eq[:], ALU.add)
            return sel

        wxsel = axis_select(px, bx, "sx", npos=3)
        wysel = axis_select(py, by, "sy")

        # fold attention weight into y-selects: ty [J, B, NP, 4Y]
        ty = a.tile([J, B, NP, 4], F32)
        nc.vector.tensor_tensor(
            ty[:], wysel[:], attw[:].unsqueeze(3).to_broadcast((J, B, NP, 4)),
            ALU.mult,
        )
        # patch weights w43 [J, 3X, B, 4Y] (x-outer for contiguous DRAM
        # bounce) = sum_n ty[n, Y] * wxsel[n, X]
        w43 = a.tile([J, 3, B, 4], F32)
        tmp43 = a.tile([J, 3, B, 4], F32)
        for n in range(NP):
            dst = (w43 if n == 0 else tmp43)[:].transpose([0, 2, 3, 1])
            nc.vector.tensor_tensor(
                dst,
                ty[:, :, n, :].unsqueeze(3).to_broadcast((J, B, 4, 3)),
                wxsel[:, :, n, :].unsqueeze(2).to_broadcast((J, B, 4, 3)),
                ALU.mult,
            )
            if n > 0:
                nc.vector.tensor_tensor(
                    w43[:].rearrange("j x b y -> j (x b y)"),
                    w43[:].rearrange("j x b y -> j (x b y)"),
                    tmp43[:].rearrange("j x b y -> j (x b y)"),
                    ALU.add,
                )

        wbc_m = bcast_weights(
            w43[:].rearrange("j x b y -> j (x b y)"), wscr_m, 48, "m",
            dest_view=lambda ap: ap.rearrange(
                "(x j c) -> j x c", x=3, j=J
            ),
            split_x=3,
        )

        # ---------------- fuse (16 chunks of 8 keypoints) ------------------
        fusedA = a.tile([128, Q, J * B // 2], BF16)
        fusedB = a.tile([128, Q, J * B // 2], BF16)
        with nc.allow_low_precision("bf16 grid-sample compute"):
            for h in range(16):
                mg = mgs[h]
                fused_half = fusedA if h < 8 else fusedB
                mv = mg[:].rearrange("p (x q) i -> p x q i", x=3)
                for x in range(3):
                    nc.vector.tensor_tensor(
                        mv[:, x, :, :],
                        mv[:, x, :, :],
                        wbc_m[:, 2048 * x + 128 * h : 2048 * x + 128 * h + 128]
                        .unsqueeze(1)
                        .to_broadcast((128, Q, MN)),
                        ALU.mult,
                    )
                # sum over x: two adds on contiguous views
                nc.vector.tensor_tensor(
                    mg[:, 0:8, :], mg[:, 0:8, :], mg[:, 8:16, :], ALU.add
                )
                nc.vector.tensor_tensor(
                    mg[:, 0:8, :], mg[:, 0:8, :], mg[:, 16:24, :], ALU.add
                )
                fsl = fused_half[:, :, 32 * (h % 8) : 32 * (h % 8) + 32]
                nc.vector.tensor_reduce(
                    fsl,
                    mg[:, 0:8, :].rearrange("p q (jb y) -> p (q jb) y", y=4),
                    AX.X,
                    ALU.add,
                )
                if h % 8 == 7:
                    # emit this half's output stage immediately so the
                    # scheduler overlaps it with the remaining fuse chunks
                    jh, fh = h // 8, fused_half
                    for b in range(B):
                        outT = op.tile([64, Q, 128], BF16, tag="outT")
                        for q in range(Q):
                            pst = ps.tile([64, 128], BF16, tag="tpb")
                            nc.tensor.transpose(
                                pst[:], fh[:, q, b::B], idb_sb[:, :J]
                            )
                            nc.scalar.copy(outT[:, q, :], pst[:])
                        nc.sync.dma_start(
                            out=out[
                                b * J + 64 * jh : b * J + 64 * (jh + 1), :
                            ].rearrange("j (q c) -> j q c", q=Q),
                            in_=outT[:],
                        )

    nc.finalize()
    return nc


def prepare_in_maps(features, keypoint_coords, w_off1, b_off1, w_off2, b_off2,
                    w_att1, b_att1, w_att2, b_att2, n_cores=8):
    bf = ml_dtypes.bfloat16
    f32 = np.float32

    def w1t(w):  # [128, C] -> [128 k_local, Q, 128 m] bf16
        return np.ascontiguousarray(
            w.T.reshape(Q, 128, 128).transpose(1, 0, 2).astype(bf)
        )

    w1o_h = w1t(np.asarray(w_off1, f32))
    w1a_h = w1t(np.asarray(w_att1, f32))
    w2o_h = np.ascontiguousarray(
        np.concatenate([w_off2[0::2], w_off2[1::2]], 0).T.astype(bf)
    )
    w2a_h = np.ascontiguousarray(np.asarray(w_att2, f32).T.astype(bf))
    b1o_h = np.asarray(b_off1, f32).reshape(128, 1).copy()
    b1a_h = np.asarray(b_att1, f32).reshape(128, 1).copy()
    b2o_h = np.concatenate([b_off2[0::2], b_off2[1::2]]).astype(f32).reshape(8, 1)
    b2a_h = np.asarray(b_att2, f32).reshape(4, 1).copy()
    bbase_h = np.broadcast_to(
        (np.arange(B, dtype=f32) * HW)[None, :], (128, B)
    ).copy()
    posc_h = np.broadcast_to(np.arange(4, dtype=f32)[None, :], (128, 4)).copy()
    ident_h = np.eye(128, dtype=f32)
    identb_h = np.eye(128, dtype=f32).astype(bf)

    in_maps = []
    for m in range(n_cores):
        bs = slice(B * m, B * (m + 1))
        feat_h = np.ascontiguousarray(
            np.asarray(features[bs], f32).transpose(0, 2, 3, 1).reshape(B * HW, C)
        ).astype(bf)
        kp_h = np.ascontiguousarray(
            np.asarray(keypoint_coords[bs], f32).transpose(1, 0, 2).reshape(J, 2 * B)
        )
        in_maps.append({
            "feat": feat_h, "kp": kp_h,
            "w1o": w1o_h, "w1a": w1a_h, "w2o": w2o_h, "w2a": w2a_h,
            "b1o": b1o_h, "b1a": b1a_h, "b2o": b2o_h, "b2a": b2a_h,
            "bbase": bbase_h, "posc": posc_h,
            "ident": ident_h, "identb": identb_h,
        })
    return in_maps


_NC_CACHE = None


def get_nc():
    global _NC_CACHE
    if _NC_CACHE is None:
        _NC_CACHE = build_nc()
    return _NC_CACHE


def kernel(**inputs):
    from concourse.bass_utils import run_bass_kernel_spmd

    n_cores = 8
    nc = get_nc()
    in_maps = prepare_in_maps(**inputs, n_cores=n_cores)
    res = run_bass_kernel_spmd(
        nc, in_maps, core_ids=list(range(n_cores)),
        trace=bool(int(os.environ.get("KERNEL_TRACE", "0") or 0)),
    )
    kernel.last_results = res
    outs = [
        np.asarray(r["out"]).astype(np.float32).reshape(B, J, C)
        for r in res.results
    ]
    return np.concatenate(outs, axis=0)



# revision 2
# speedup vs baseline: 1.2372x; 1.2372x over previous
"""Trainium2 Bass kernel for nn_AdaptiveSampler (sparse grid_sample attention).

Strategy v2 (data-parallel over batch, 8 cores x 4 batch items each):
  - Host: features reshaped channels-last [B*H*W, C] bf16 (2KB rows), and
    because ALL gather indices depend only on keypoint_coords (not on the
    device-computed MLP offsets), the host precomputes per-keypoint 3x3
    patch row indices (in the gpsimd wrapped int16 layout) plus the seed
    bilinear select-weights.
  - Device per core:
      one 3x3-cell patch gather per batch item (keypoint-major: each
      patch row lands on the keypoint's partition, [128j, 3y, 3x*1024c]).
      seed   = sum_xy ws9[j,xy] * patch[j,xy,:]  on DVE (per-partition
               weight broadcast; the 4 seed bilinear corners are a subset
               of the 3x3 patch, so no separate seed gather at all)
      MLPs   = PE matmuls after 8 PE transposes of seed to channel-major
      fuse   = 9 diagonal-matmuls per batch item on PE: stationary
               diag(w9[:,xy]) (bf16), moving = patch x-slices, accumulated
               in PSUM f32 -> output lands directly keypoint-major [j, C],
               so no output transposes and no weight broadcast DMA.
All computation happens on-device except index/layout prep on host.
"""

import os
import sys
from contextlib import ExitStack

import numpy as np

sys.path.insert(0, "/opt/trn_rl_repo")

import ml_dtypes

import concourse.bass as bass
import concourse.tile as tile
from concourse import bacc, mybir

F32 = mybir.dt.float32
BF16 = mybir.dt.bfloat16
I16 = mybir.dt.int16

ALU = mybir.AluOpType
ACT = mybir.ActivationFunctionType
AX = mybir.AxisListType

B = 4          # batch items per core
C = 1024       # channels
H = W = 64
HW = H * W     # 4096 cells per batch item
J = 128        # keypoints
NP = 4         # sample points per keypoint
Q = C // 128   # 8 channel chunks
TWO23 = float(2 ** 23)


def _floor(nc, pool, src, shape, tag):
    """floor(src) on DVE via round-to-nearest + correction. Returns tile."""
    rnd = pool.tile(list(shape), F32, tag=f"floor_rnd_{tag}")
    nc.vector.tensor_scalar(rnd[:], src, TWO23, TWO23, ALU.add, ALU.subtract)
    flo = pool.tile(list(shape), F32, tag=f"floor_out_{tag}")
    nc.vector.tensor_tensor(flo[:], src, rnd[:], ALU.is_lt)
    nc.vector.tensor_tensor(flo[:], rnd[:], flo[:], ALU.subtract)
    return flo


def build_nc():
    nc = bacc.Bacc()

    feat = nc.declare_dram_parameter("feat", [B * HW, C], BF16, isOutput=False)
    idxg = nc.declare_dram_parameter("idxg", [128, B * 24], I16, isOutput=False)
    ws9 = nc.declare_dram_parameter("ws9", [128, B * 9], BF16, isOutput=False)
    coef = nc.declare_dram_parameter("coef", [128, 16], F32, isOutput=False)
    w1o = nc.declare_dram_parameter("w1o", [128, Q, 128], BF16, isOutput=False)
    w1a = nc.declare_dram_parameter("w1a", [128, Q, 128], BF16, isOutput=False)
    w2o = nc.declare_dram_parameter("w2o", [128, 8], BF16, isOutput=False)
    w2a = nc.declare_dram_parameter("w2a", [128, 4], BF16, isOutput=False)
    b1o = nc.declare_dram_parameter("b1o", [128, 1], F32, isOutput=False)
    b1a = nc.declare_dram_parameter("b1a", [128, 1], F32, isOutput=False)
    b2o = nc.declare_dram_parameter("b2o", [8, 1], F32, isOutput=False)
    b2a = nc.declare_dram_parameter("b2a", [4, 1], F32, isOutput=False)
    posc = nc.declare_dram_parameter("posc", [128, 4], F32, isOutput=False)
    identb = nc.declare_dram_parameter("identb", [128, 128], BF16, isOutput=False)
    out = nc.declare_dram_parameter("out", [B * J, C], BF16, isOutput=True)

    # Overlapping row view of feat: row i = cells i..i+2 (6KB). Max row start
    # 16381 -> read end == tensor end exactly.
    feat_ov = bass.AP(feat[:].tensor, 0, [[C, B * HW - 2], [1, 3 * C]])

    with ExitStack() as ctx:
        tc = ctx.enter_context(tile.TileContext(nc))
        cons = ctx.enter_context(tc.tile_pool(name="cons", bufs=1))
        gp = ctx.enter_context(tc.tile_pool(name="gpool", bufs=1))
        a = ctx.enter_context(tc.tile_pool(name="work", bufs=1))
        dgp = ctx.enter_context(tc.tile_pool(name="diag", bufs=2))
        ps = ctx.enter_context(tc.tile_pool(name="psT", bufs=2, space="PSUM"))
        pmm = ctx.enter_context(tc.tile_pool(name="psMM", bufs=2, space="PSUM"))
        pfu = ctx.enter_context(tc.tile_pool(name="psFU", bufs=3, space="PSUM"))

        # ---------------- constants ----------------
        def c_load(name, shape, dt, src):
            t = cons.tile(shape, dt, tag=name)
            nc.sync.dma_start(out=t[:], in_=src)
            return t

        idxg_sb = c_load("idxg", [128, B * 24], I16, idxg[:])
        ws9_sb = c_load("ws9", [128, B * 9], BF16, ws9[:])
        coef_sb = c_load("coef", [128, 16], F32, coef[:])
        w1o_sb = c_load("w1o", [128, Q, 128], BF16, w1o[:])
        w1a_sb = c_load("w1a", [128, Q, 128], BF16, w1a[:])
        w2o_sb = c_load("w2o", [128, 8], BF16, w2o[:])
        w2a_sb = c_load("w2a", [128, 4], BF16, w2a[:])
        b1o_sb = c_load("b1o", [128, 1], F32, b1o[:])
        b1a_sb = c_load("b1a", [128, 1], F32, b1a[:])
        b2o_sb = c_load("b2o", [8, 1], F32, b2o[:])
        b2a_sb = c_load("b2a", [4, 1], F32, b2a[:])
        posc_sb = c_load("posc", [128, 4], F32, posc[:])
        idb_sb = c_load("identb", [128, 128], BF16, identb[:])

        ixv = coef_sb[:, 0:4]    # [J, B] pixel x coords
        iyv = coef_sb[:, 4:8]
        bxv = coef_sb[:, 8:12]   # patch x base (f32 integer-valued)
        byv = coef_sb[:, 12:16]

        # ---------------- patch gathers (one per batch item) --------------
        Gt = []
        for b in range(B):
            g = gp.tile([128, 3, 3 * C], BF16, tag=f"G{b}")
            nc.gpsimd.dma_gather(
                g[:],
                feat_ov,
                idxg_sb[:, b * 24 : (b + 1) * 24],
                num_idxs=3 * J,
                num_idxs_reg=3 * J,
                elem_size=3 * C,
                elem_step=C,
                transpose=False,
            )
            Gt.append(g)

        # ---------------- seed combine (DVE, keypoint-major) --------------
        seedcm = a.tile([128, Q, B, 128], BF16)
        with nc.allow_low_precision("bf16 grid-sample compute"):
            for b in range(B):
                sjb = a.tile([128, C], BF16, tag=f"sjb{b}")
                stmp = a.tile([128, C], BF16, tag="stmp")
                k = 0
                for y in range(3):
                    for x in range(3):
                        wsl = ws9_sb[
                            :, b * 9 + k : b * 9 + k + 1
                        ].to_broadcast((128, C))
                        gsl = Gt[b][:, y, x * C : (x + 1) * C]
                        if k == 0:
                            nc.vector.tensor_tensor(sjb[:], gsl, wsl, ALU.mult)
                        else:
                            nc.vector.tensor_tensor(stmp[:], gsl, wsl, ALU.mult)
                            nc.vector.tensor_tensor(
                                sjb[:], sjb[:], stmp[:], ALU.add
                            )
                        k += 1
                # transpose seed to channel-major [128c, q, b, 128j]
                for q in range(Q):
                    pst = ps.tile([128, 128], BF16, tag="tp")
                    nc.tensor.transpose(
                        pst[:], sjb[:, q * 128 : (q + 1) * 128], idb_sb[:]
                    )
                    nc.scalar.copy(seedcm[:, q, b, :], pst[:])

        # ---------------- MLPs (PE) ----------------
        mov = seedcm[:].rearrange("p q b j -> p q (b j)")

        def mlp_head(w1_sb, b1_sb, name):
            hps = pmm.tile([128, J * B], F32, tag="mlp")
            for q in range(Q):
                nc.tensor.matmul(
                    hps[:],
                    w1_sb[:, q, :],
                    mov[:, q, :],
                    start=(q == 0),
                    stop=(q == Q - 1),
                )
            h_sb = a.tile([128, J * B], BF16, tag=f"hsb_{name}")
            nc.scalar.activation(h_sb[:], hps[:], ACT.Relu, bias=b1_sb[:])
            return h_sb

        h_off = mlp_head(w1o_sb, b1o_sb, "off")
        h_att = mlp_head(w1a_sb, b1a_sb, "att")

        ops2 = pmm.tile([8, J * B], F32, tag="mlp")
        nc.tensor.matmul(ops2[:], w2o_sb[:], h_off[:], start=True, stop=True)
        off2 = a.tile([8, J * B], BF16)
        nc.scalar.activation(off2[:], ops2[:], ACT.Identity, bias=b2o_sb[:])

        aps2 = pmm.tile([4, J * B], F32, tag="mlp")
        nc.tensor.matmul(aps2[:], w2a_sb[:], h_att[:], start=True, stop=True)
        att2 = a.tile([4, J * B], BF16)
        nc.scalar.activation(att2[:], aps2[:], ACT.Identity, bias=b2a_sb[:])

        # transpose MLP outputs back to [J, B, ch] (contiguous b-blocks)
        offT = a.tile([J, B, 8], F32)
        attT = a.tile([J, B, 4], F32)
        for b in range(B):
            pso = ps.tile([128, 8], BF16, tag="tp")
            nc.tensor.transpose(
                pso[:, 0:8], off2[:, b * J : (b + 1) * J], idb_sb[:8, :8]
            )
            nc.scalar.copy(offT[:, b, :], pso[:, 0:8])
            psa = ps.tile([128, 4], BF16, tag="tp")
            nc.tensor.transpose(
                psa[:, 0:4], att2[:, b * J : (b + 1) * J], idb_sb[:4, :4]
            )
            nc.scalar.copy(attT[:, b, :], psa[:, 0:4])

        # ---------------- fuse weights (DVE, batched [J, B, ...]) ---------
        px = a.tile([J, B, NP], F32)
        nc.vector.tensor_tensor(
            px[:],
            ixv.unsqueeze(2).to_broadcast((J, B, NP)),
            offT[:, :, 0:NP],
            ALU.add,
        )
        py = a.tile([J, B, NP], F32)
        nc.vector.tensor_tensor(
            py[:],
            iyv.unsqueeze(2).to_broadcast((J, B, NP)),
            offT[:, :, NP : 2 * NP],
            ALU.add,
        )

        # softmax over NP  [J, B, NP]
        amax = a.tile([J, B, 1], F32)
        nc.vector.tensor_reduce(amax[:], attT[:], AX.X, ALU.max)
        ae = a.tile([J, B, NP], F32)
        nc.vector.tensor_tensor(
            ae[:], attT[:], amax[:].to_broadcast((J, B, NP)), ALU.subtract
        )
        nc.scalar.activation(ae[:], ae[:], ACT.Exp)
        asum = a.tile([J, B, 1], F32)
        nc.vector.tensor_reduce(asum[:], ae[:], AX.X, ALU.add)
        nc.vector.reciprocal(asum[:], asum[:])
        attw = a.tile([J, B, NP], F32)
        nc.vector.tensor_tensor(
            attw[:], ae[:], asum[:].to_broadcast((J, B, NP)), ALU.mult
        )

        def axis_select(pc, base, tagp):
            """Position-select weights [J, B, NP, 3]:
            w0*(pos==d) + w1*(pos==d+1), d = floor(pc) - base."""
            c0 = _floor(nc, a, pc[:], (J, B, NP), tagp)
            w1t = a.tile([J, B, NP], F32, tag=f"{tagp}_w1")
            nc.vector.tensor_tensor(w1t[:], pc[:], c0[:], ALU.subtract)
            w0t = a.tile([J, B, NP], F32, tag=f"{tagp}_w0")
            nc.vector.tensor_scalar(w0t[:], w1t[:], -1.0, 1.0, ALU.mult, ALU.add)
            d = a.tile([J, B, NP], F32, tag=f"{tagp}_d")
            nc.vector.tensor_tensor(
                d[:], c0[:], base.unsqueeze(2).to_broadcast((J, B, NP)),
                ALU.subtract,
            )
            d1 = a.tile([J, B, NP], F32, tag=f"{tagp}_d1")
            nc.vector.tensor_scalar_add(d1[:], d[:], 1.0)
            posb = (
                posc_sb[:, 0:3]
                .unsqueeze(1)
                .unsqueeze(2)
                .to_broadcast((J, B, NP, 3))
            )
            sel = a.tile([J, B, NP, 3], F32, tag=f"{tagp}_sel")
            eq = a.tile([J, B, NP, 3], F32, tag=f"{tagp}_eq")
            nc.vector.tensor_tensor(
                eq[:], d[:].unsqueeze(3).to_broadcast((J, B, NP, 3)), posb,
                ALU.is_equal,
            )
            nc.vector.tensor_tensor(
                sel[:], eq[:], w0t[:].unsqueeze(3).to_broadcast((J, B, NP, 3)),
                ALU.mult,
            )
            nc.vector.tensor_tensor(
                eq[:], d1[:].unsqueeze(3).to_broadcast((J, B, NP, 3)), posb,
                ALU.is_equal,
            )
            nc.vector.tensor_tensor(
                eq[:], eq[:], w1t[:].unsqueeze(3).to_broadcast((J, B, NP, 3)),
                ALU.mult,
            )
            nc.vector.tensor_tensor(sel[:], sel[:], eq[:], ALU.add)
            return sel

        wxsel = axis_select(px, bxv, "sx")
        wysel = axis_select(py, byv, "sy")

        # fold attention weight into y-selects: ty [J, B, NP, 3Y]
        ty = a.tile([J, B, NP, 3], F32)
        nc.vector.tensor_tensor(
            ty[:], wysel[:], attw[:].unsqueeze(3).to_broadcast((J, B, NP, 3)),
            ALU.mult,
        )
        # per-cell weights w9 [J, B, 3Y, 3X] = sum_n ty[n, Y] * wxsel[n, X]
        w9 = a.tile([J, B, 3, 3], F32)
        tmp9 = a.tile([J, B, 3, 3], F32)
        for n in range(NP):
            dst = (w9 if n == 0 else tmp9)
            nc.vector.tensor_tensor(
                dst[:],
                ty[:, :, n, :].unsqueeze(3).to_broadcast((J, B, 3, 3)),
                wxsel[:, :, n, :].unsqueeze(2).to_broadcast((J, B, 3, 3)),
                ALU.mult,
            )
            if n > 0:
                nc.vector.tensor_tensor(w9[:], w9[:], tmp9[:], ALU.add)
        w9b = a.tile([J, B, 9], BF16)
        nc.vector.tensor_copy(w9b[:], w9[:].rearrange("j b y x -> j b (y x)"))

        # ---------------- fuse (PE diagonal matmuls, per batch item) ------
        for b in range(B):
            dgs = []
            for k in range(9):
                dg = dgp.tile([128, 128], BF16, tag=f"dg{k}")
                nc.vector.tensor_tensor(
                    dg[:],
                    idb_sb[:],
                    w9b[:, b, k : k + 1].to_broadcast((128, 128)),
                    ALU.mult,
                )
                dgs.append(dg)
            fo = a.tile([128, C], BF16, tag=f"fo{b}")
            for hh in range(2):
                acc = pfu.tile([128, 512], F32, tag="facc")
                k = 0
                for y in range(3):
                    for x in range(3):
                        nc.tensor.matmul(
                            acc[:],
                            dgs[k][:],
                            Gt[b][:, y, x * C + hh * 512 : x * C + hh * 512 + 512],
                            start=(k == 0),
                            stop=(k == 8),
                        )
                        k += 1
                nc.scalar.copy(fo[:, hh * 512 : (hh + 1) * 512], acc[:])
            nc.sync.dma_start(out=out[b * J : (b + 1) * J, :], in_=fo[:])

    nc.finalize()
    return nc


def prepare_in_maps(features, keypoint_coords, w_off1, b_off1, w_off2, b_off2,
                    w_att1, b_att1, w_att2, b_att2, n_cores=8):
    bf = ml_dtypes.bfloat16
    f32 = np.float32

    def w1t(w):  # [128, C] -> [128 k_local, Q, 128 m] bf16
        return np.ascontiguousarray(
            w.T.reshape(Q, 128, 128).transpose(1, 0, 2).astype(bf)
        )

    w1o_h = w1t(np.asarray(w_off1, f32))
    w1a_h = w1t(np.asarray(w_att1, f32))
    w2o_h = np.ascontiguousarray(
        np.concatenate([w_off2[0::2], w_off2[1::2]], 0).T.astype(bf)
    )
    w2a_h = np.ascontiguousarray(np.asarray(w_att2, f32).T.astype(bf))
    b1o_h = np.asarray(b_off1, f32).reshape(128, 1).copy()
    b1a_h = np.asarray(b_att1, f32).reshape(128, 1).copy()
    b2o_h = np.concatenate([b_off2[0::2], b_off2[1::2]]).astype(f32).reshape(8, 1)
    b2a_h = np.asarray(b_att2, f32).reshape(4, 1).copy()
    posc_h = np.broadcast_to(np.arange(4, dtype=f32)[None, :], (128, 4)).copy()
    identb_h = np.eye(128, dtype=f32).astype(bf)

    # host-side keypoint geometry (all gather indices + seed weights derive
    # from keypoint_coords only)
    kp = np.asarray(keypoint_coords, f32)           # [32, J, 2]
    ix = (kp[..., 0] + 1.0) * 31.5                  # [32, J]
    iy = (kp[..., 1] + 1.0) * 31.5
    x0 = np.floor(ix); y0 = np.floor(iy)
    fx = ix - x0; fy = iy - y0
    bx = np.clip(np.round(ix) - 1.0, 0.0, 61.0)
    by = np.clip(np.round(iy) - 1.0, 0.0, 61.0)
    dx = x0 - bx                                    # in {0, 1}
    dy = y0 - by
    pos3 = np.arange(3, dtype=f32)
    wsx = ((1.0 - fx)[..., None] * (pos3 == dx[..., None])
           + fx[..., None] * (pos3 == dx[..., None] + 1.0))   # [32, J, 3]
    wsy = ((1.0 - fy)[..., None] * (pos3 == dy[..., None])
           + fy[..., None] * (pos3 == dy[..., None] + 1.0))
    ws9_all = wsy[..., :, None] * wsx[..., None, :]           # [32, J, 3y, 3x]
    rowidx = ((by[..., None] + pos3) * 64.0 + bx[..., None])  # [32, J, 3y]

    in_maps = []
    for m in range(n_cores):
        bs = slice(B * m, B * (m + 1))
        feat_h = np.ascontiguousarray(
            np.asarray(features[bs], f32).transpose(0, 2, 3, 1).reshape(B * HW, C)
        ).astype(bf)
        # gather idx: per b, flat order i = y*J + j; wrapped (p, c) layout
        idxg_h = np.empty((128, B * 24), np.int16)
        for b in range(B):
            gb = B * m + b
            flat = (rowidx[gb].T.reshape(3 * J) + b * HW).astype(np.int16)
            idxg_h[:, b * 24 : (b + 1) * 24] = np.tile(
                flat.reshape(24, 16).T, (8, 1)
            )
        ws9_h = np.ascontiguousarray(
            ws9_all[bs].transpose(1, 0, 2, 3).reshape(J, B * 9).astype(bf)
        )
        coef_h = np.ascontiguousarray(
            np.concatenate(
                [ix[bs].T, iy[bs].T, bx[bs].T, by[bs].T], axis=1
            ).astype(f32)
        )
        in_maps.append({
            "feat": feat_h, "idxg": idxg_h, "ws9": ws9_h, "coef": coef_h,
            "w1o": w1o_h, "w1a": w1a_h, "w2o": w2o_h, "w2a": w2a_h,
            "b1o": b1o_h, "b1a": b1a_h, "b2o": b2o_h, "b2a": b2a_h,
            "posc": posc_h, "identb": identb_h,
        })
    return in_maps


_NC_CACHE = None


def get_nc():
    global _NC_CACHE
    if _NC_CACHE is None:
        _NC_CACHE = build_nc()
    return _NC_CACHE


def kernel(**inputs):
    from concourse.bass_utils import run_bass_kernel_spmd

    n_cores = 8
    nc = get_nc()
    in_maps = prepare_in_maps(**inputs, n_cores=n_cores)
    res = run_bass_kernel_spmd(
        nc, in_maps, core_ids=list(range(n_cores)),
        trace=bool(int(os.environ.get("KERNEL_TRACE", "0") or 0)),
    )
    kernel.last_results = res
    outs = [
        np.asarray(r["out"]).astype(np.float32).reshape(B, J, C)
        for r in res.results
    ]
    return np.concatenate(outs, axis=0)


# revision 6
# speedup vs baseline: 2.0097x; 1.6244x over previous
"""Trainium2 Bass kernel for nn_AdaptiveSampler (sparse grid_sample attention).

Strategy v3 (data-parallel over batch, 8 cores x 4 batch items each):
  - Host: features channels-last [B*H*W, C] bf16 (2KB rows). All gather
    indices depend only on keypoint_coords, so the host precomputes them
    in the gpsimd wrapped int16 layout, plus the per-column seed bilinear
    weights.
  - Device per core:
      seed: ONE transpose-gather of 2x2 corner cells as 2-cell rows
            (channel-major [128c, (x,q), (y,jb)]), then DVE multiply by
            per-column corner weights (partition-broadcast, 2x bf16 rate)
            and two halving adds -> seed lands directly channel-major,
            feeding the MLP matmuls with no PE transposes.
      patch: one 3x3-cell gather per batch item, keypoint-major
            ([128j, 3y, 3x*1024c]) for the fuse stage.
      MLPs: PE matmuls; offsets/attention transposed back per-b (PE).
      fuse: 9 diagonal-matmuls per batch item on PE: stationary
            diag(w9[:,xy]) built on the Scalar engine (activation with
            per-partition scale), moving = patch x-slices, accumulated in
            PSUM f32 -> output lands keypoint-major [j, C]; direct DMA out.
"""

import os
import sys
from contextlib import ExitStack

import numpy as np

sys.path.insert(0, "/opt/trn_rl_repo")

import ml_dtypes

import concourse.bass as bass
import concourse.tile as tile
from concourse import bacc, mybir

F32 = mybir.dt.float32
BF16 = mybir.dt.bfloat16
I16 = mybir.dt.int16

ALU = mybir.AluOpType
ACT = mybir.ActivationFunctionType
AX = mybir.AxisListType

B = 4          # batch items per core
C = 1024       # channels
H = W = 64
HW = H * W     # 4096 cells per batch item
J = 128        # keypoints
NP = 4         # sample points per keypoint
Q = C // 128   # 8 channel chunks
TWO23 = float(2 ** 23)


def _floor(nc, pool, src, shape, tag):
    """floor(src) on DVE via round-to-nearest + correction. Returns tile."""
    rnd = pool.tile(list(shape), F32, tag=f"floor_rnd_{tag}")
    nc.vector.tensor_scalar(rnd[:], src, TWO23, TWO23, ALU.add, ALU.subtract)
    flo = pool.tile(list(shape), F32, tag=f"floor_out_{tag}")
    nc.vector.tensor_tensor(flo[:], src, rnd[:], ALU.is_lt)
    nc.vector.tensor_tensor(flo[:], rnd[:], flo[:], ALU.subtract)
    return flo


def build_nc():
    nc = bacc.Bacc()

    feat = nc.declare_dram_parameter("feat", [B * HW, C], BF16, isOutput=False)
    # seed idx [128, 64] ++ per-b patch idx [128, 24] each
    idxg = nc.declare_dram_parameter("idxg", [128, 64 + B * 24], I16,
                                     isOutput=False)
    wsg = nc.declare_dram_parameter("wsg", [2 * 2 * J * B], BF16, isOutput=False)
    coef = nc.declare_dram_parameter("coef", [128, 16], F32, isOutput=False)
    w1o = nc.declare_dram_parameter("w1o", [128, Q, 128], BF16, isOutput=False)
    w1a = nc.declare_dram_parameter("w1a", [128, Q, 128], BF16, isOutput=False)
    w2o = nc.declare_dram_parameter("w2o", [128, 8], BF16, isOutput=False)
    w2a = nc.declare_dram_parameter("w2a", [128, 4], BF16, isOutput=False)
    b1o = nc.declare_dram_parameter("b1o", [128, 1], F32, isOutput=False)
    b1a = nc.declare_dram_parameter("b1a", [128, 1], F32, isOutput=False)
    b2o = nc.declare_dram_parameter("b2o", [8, 1], F32, isOutput=False)
    b2a = nc.declare_dram_parameter("b2a", [4, 1], F32, isOutput=False)
    posc = nc.declare_dram_parameter("posc", [128, 4], F32, isOutput=False)
    identb = nc.declare_dram_parameter("identb", [128, 128], BF16, isOutput=False)
    out = nc.declare_dram_parameter("out", [B * J, C], BF16, isOutput=True)

    # Overlapping row views of feat. 3-cell rows (patch): max start 16381.
    # 2-cell rows (seed): max start 16382. Read end == tensor end exactly.
    feat_ov3 = bass.AP(feat[:].tensor, 0, [[C, B * HW - 2], [1, 3 * C]])
    feat_ov2 = bass.AP(feat[:].tensor, 0, [[C, B * HW - 1], [1, 2 * C]])

    with ExitStack() as ctx:
        tc = ctx.enter_context(tile.TileContext(nc))
        cons = ctx.enter_context(tc.tile_pool(name="cons", bufs=1))
        gp = ctx.enter_context(tc.tile_pool(name="gpool", bufs=1))
        a = ctx.enter_context(tc.tile_pool(name="work", bufs=1))
        dgp = ctx.enter_context(tc.tile_pool(name="diag", bufs=2))
        ps = ctx.enter_context(tc.tile_pool(name="psT", bufs=2, space="PSUM"))
        pmm = ctx.enter_context(tc.tile_pool(name="psMM", bufs=2, space="PSUM"))
        pfu = ctx.enter_context(tc.tile_pool(name="psFU", bufs=3, space="PSUM"))

        # ---------------- constants ----------------
        def c_load(name, shape, dt, src):
            t = cons.tile(shape, dt, tag=name)
            nc.sync.dma_start(out=t[:], in_=src)
            return t

        idxg_sb = c_load("idxg", [128, 64 + B * 24], I16, idxg[:])
        coef_sb = c_load("coef", [128, 16], F32, coef[:])
        w1o_sb = c_load("w1o", [128, Q, 128], BF16, w1o[:])
        w1a_sb = c_load("w1a", [128, Q, 128], BF16, w1a[:])
        w2o_sb = c_load("w2o", [128, 8], BF16, w2o[:])
        w2a_sb = c_load("w2a", [128, 4], BF16, w2a[:])
        b1o_sb = c_load("b1o", [128, 1], F32, b1o[:])
        b1a_sb = c_load("b1a", [128, 1], F32, b1a[:])
        b2o_sb = c_load("b2o", [8, 1], F32, b2o[:])
        b2a_sb = c_load("b2a", [4, 1], F32, b2a[:])
        posc_sb = c_load("posc", [128, 4], F32, posc[:])
        idb_sb = c_load("identb", [128, 128], BF16, identb[:])
        wsg_sb = cons.tile([128, 2, 2, J * B], BF16, tag="wsg")
        nc.sync.dma_start(
            out=wsg_sb[:],
            in_=wsg[:].unsqueeze(0).to_broadcast((128, 2 * 2 * J * B)),
        )

        ixv = coef_sb[:, 0:4]    # [J, B] pixel x coords
        iyv = coef_sb[:, 4:8]
        bxv = coef_sb[:, 8:12]   # patch x base (f32 integer-valued)
        byv = coef_sb[:, 12:16]

        # ---------------- seed gather (channel-major, 4 chunks) -----------
        # chunk h: y = h//2, jb half = h%2 (256 idxs each)
        G2h = []
        for h in range(4):
            g2 = gp.tile([128, 16, 256], BF16, tag=f"G2{h}")
            nc.gpsimd.dma_gather(
                g2[:],
                feat_ov2,
                idxg_sb[:, 16 * h : 16 * h + 16],
                num_idxs=256,
                num_idxs_reg=256,
                elem_size=2 * C,
                elem_step=C,
                transpose=True,
            )
            G2h.append(g2)

        # ---------------- patch gathers (keypoint-major, one per b) -------
        Gt = []
        for b in range(B):
            g = gp.tile([128, 3, 3 * C], BF16, tag=f"G{b}")
            nc.gpsimd.dma_gather(
                g[:],
                feat_ov3,
                idxg_sb[:, 64 + b * 24 : 64 + (b + 1) * 24],
                num_idxs=3 * J,
                num_idxs_reg=3 * J,
                elem_size=3 * C,
                elem_step=C,
                transpose=False,
            )
            Gt.append(g)

        # ---------------- seed combine (DVE, 2x-rate contiguous ops) ------
        seed = a.tile([128, Q, J * B], BF16)
        with nc.allow_low_precision("bf16 grid-sample compute"):
            for h in range(4):
                y, jh = h // 2, h % 2
                g2 = G2h[h]
                # [128, x2, q8, 256jb] *= wsg[x, y(h), jb-slice]
                g2v = g2[:].rearrange("p (x q) i -> p x q i", x=2)
                nc.vector.tensor_tensor(
                    g2v,
                    g2v,
                    wsg_sb[:, :, y, 256 * jh : 256 * jh + 256]
                    .unsqueeze(2)
                    .to_broadcast((128, 2, Q, 256)),
                    ALU.mult,
                )
                # x-add: first half += second half (contiguous)
                g2f = g2[:].rearrange("p e i -> p (e i)")
                nc.vector.tensor_tensor(
                    g2f[0:128, 0 : Q * 256],
                    g2f[0:128, 0 : Q * 256],
                    g2f[0:128, Q * 256 : 2 * Q * 256],
                    ALU.add,
                )
            # y-add into contiguous seed tile, per jb half
            for jh in range(2):
                nc.vector.tensor_tensor(
                    seed[:, :, 256 * jh : 256 * jh + 256],
                    G2h[jh][:, 0:Q, :],
                    G2h[2 + jh][:, 0:Q, :],
                    ALU.add,
                )

        # ---------------- MLPs (PE) ----------------
        def mlp_head(w1_sb, b1_sb, name):
            hps = pmm.tile([128, J * B], F32, tag="mlp")
            for q in range(Q):
                nc.tensor.matmul(
                    hps[:],
                    w1_sb[:, q, :],
                    seed[:, q, :],
                    start=(q == 0),
                    stop=(q == Q - 1),
                )
            h_sb = a.tile([128, J * B], BF16, tag=f"hsb_{name}")
            nc.scalar.activation(h_sb[:], hps[:], ACT.Relu, bias=b1_sb[:])
            return h_sb

        h_off = mlp_head(w1o_sb, b1o_sb, "off")
        h_att = mlp_head(w1a_sb, b1a_sb, "att")

        ops2 = pmm.tile([8, J * B], F32, tag="mlp")
        nc.tensor.matmul(ops2[:], w2o_sb[:], h_off[:], start=True, stop=True)
        off2 = a.tile([8, J * B], BF16)
        nc.scalar.activation(off2[:], ops2[:], ACT.Identity, bias=b2o_sb[:])

        aps2 = pmm.tile([4, J * B], F32, tag="mlp")
        nc.tensor.matmul(aps2[:], w2a_sb[:], h_att[:], start=True, stop=True)
        att2 = a.tile([4, J * B], BF16)
        nc.scalar.activation(att2[:], aps2[:], ACT.Identity, bias=b2a_sb[:])

        # transpose MLP outputs back to [J, B, ch] (contiguous b-blocks)
        offT = a.tile([J, B, 8], F32)
        attT = a.tile([J, B, 4], F32)
        for b in range(B):
            pso = ps.tile([128, 8], BF16, tag="tp")
            nc.tensor.transpose(
                pso[:, 0:8], off2[:, b * J : (b + 1) * J], idb_sb[:8, :8]
            )
            nc.scalar.copy(offT[:, b, :], pso[:, 0:8])
            psa = ps.tile([128, 4], BF16, tag="tp")
            nc.tensor.transpose(
                psa[:, 0:4], att2[:, b * J : (b + 1) * J], idb_sb[:4, :4]
            )
            nc.scalar.copy(attT[:, b, :], psa[:, 0:4])

        # ---------------- fuse weights (DVE, batched [J, B, ...]) ---------
        px = a.tile([J, B, NP], F32)
        nc.vector.tensor_tensor(
            px[:],
            ixv.unsqueeze(2).to_broadcast((J, B, NP)),
            offT[:, :, 0:NP],
            ALU.add,
        )
        py = a.tile([J, B, NP], F32)
        nc.vector.tensor_tensor(
            py[:],
            iyv.unsqueeze(2).to_broadcast((J, B, NP)),
            offT[:, :, NP : 2 * NP],
            ALU.add,
        )

        # softmax over NP  [J, B, NP]
        amax = a.tile([J, B, 1], F32)
        nc.vector.tensor_reduce(amax[:], attT[:], AX.X, ALU.max)
        ae = a.tile([J, B, NP], F32)
        nc.vector.tensor_tensor(
            ae[:], attT[:], amax[:].to_broadcast((J, B, NP)), ALU.subtract
        )
        nc.scalar.activation(ae[:], ae[:], ACT.Exp)
        asum = a.tile([J, B, 1], F32)
        nc.vector.tensor_reduce(asum[:], ae[:], AX.X, ALU.add)
        nc.vector.reciprocal(asum[:], asum[:])
        attw = a.tile([J, B, NP], F32)
        nc.vector.tensor_tensor(
            attw[:], ae[:], asum[:].to_broadcast((J, B, NP)), ALU.mult
        )

        def axis_select(pc, base, tagp):
            """Position-select weights [J, B, NP, 3]:
            w0*(pos==d) + w1*(pos==d+1), d = floor(pc) - base."""
            c0 = _floor(nc, a, pc[:], (J, B, NP), tagp)
            w1t = a.tile([J, B, NP], F32, tag=f"{tagp}_w1")
            nc.vector.tensor_tensor(w1t[:], pc[:], c0[:], ALU.subtract)
            w0t = a.tile([J, B, NP], F32, tag=f"{tagp}_w0")
            nc.vector.tensor_scalar(w0t[:], w1t[:], -1.0, 1.0, ALU.mult, ALU.add)
            d = a.tile([J, B, NP], F32, tag=f"{tagp}_d")
            nc.vector.tensor_tensor(
                d[:], c0[:], base.unsqueeze(2).to_broadcast((J, B, NP)),
                ALU.subtract,
            )
            d1 = a.tile([J, B, NP], F32, tag=f"{tagp}_d1")
            nc.vector.tensor_scalar_add(d1[:], d[:], 1.0)
            posb = (
                posc_sb[:, 0:3]
                .unsqueeze(1)
                .unsqueeze(2)
                .to_broadcast((J, B, NP, 3))
            )
            sel = a.tile([J, B, NP, 3], F32, tag=f"{tagp}_sel")
            eq = a.tile([J, B, NP, 3], F32, tag=f"{tagp}_eq")
            nc.vector.tensor_tensor(
                eq[:], d[:].unsqueeze(3).to_broadcast((J, B, NP, 3)), posb,
                ALU.is_equal,
            )
            nc.vector.tensor_tensor(
                sel[:], eq[:], w0t[:].unsqueeze(3).to_broadcast((J, B, NP, 3)),
                ALU.mult,
            )
            nc.vector.tensor_tensor(
                eq[:], d1[:].unsqueeze(3).to_broadcast((J, B, NP, 3)), posb,
                ALU.is_equal,
            )
            nc.vector.tensor_tensor(
                eq[:], eq[:], w1t[:].unsqueeze(3).to_broadcast((J, B, NP, 3)),
                ALU.mult,
            )
            nc.vector.tensor_tensor(sel[:], sel[:], eq[:], ALU.add)
            return sel

        wxsel = axis_select(px, bxv, "sx")
        wysel = axis_select(py, byv, "sy")

        # fold attention weight into y-selects: ty [J, B, NP, 3Y]
        ty = a.tile([J, B, NP, 3], F32)
        nc.vector.tensor_tensor(
            ty[:], wysel[:], attw[:].unsqueeze(3).to_broadcast((J, B, NP, 3)),
            ALU.mult,
        )
        # per-cell weights w9 [J, B, 3Y, 3X] = sum_n ty[n, Y] * wxsel[n, X]
        w9 = a.tile([J, B, 3, 3], F32)
        tmp9 = a.tile([J, B, 3, 3], F32)
        for n in range(NP):
            dst = (w9 if n == 0 else tmp9)
            nc.vector.tensor_tensor(
                dst[:],
                ty[:, :, n, :].unsqueeze(3).to_broadcast((J, B, 3, 3)),
                wxsel[:, :, n, :].unsqueeze(2).to_broadcast((J, B, 3, 3)),
                ALU.mult,
            )
            if n > 0:
                nc.vector.tensor_tensor(w9[:], w9[:], tmp9[:], ALU.add)

        # ---------------- fuse (PE diagonal matmuls, per batch item) ------
        for b in range(B):
            dgs = []
            for y in range(3):
                for x in range(3):
                    dg = dgp.tile([128, 128], BF16, tag=f"dg{y}{x}")
                    nc.scalar.activation(
                        dg[:], idb_sb[:], ACT.Identity,
                        scale=w9[:, b, y, x : x + 1],
                    )
                    dgs.append(dg)
            fo = a.tile([128, C], BF16, tag=f"fo{b}")
            for hh in range(2):
                acc = pfu.tile([128, 512], F32, tag="facc")
                k = 0
                for y in range(3):
                    for x in range(3):
                        nc.tensor.matmul(
                            acc[:],
                            dgs[k][:],
                            Gt[b][:, y, x * C + hh * 512 : x * C + hh * 512 + 512],
                            start=(k == 0),
                            stop=(k == 8),
                        )
                        k += 1
                nc.scalar.copy(fo[:, hh * 512 : (hh + 1) * 512], acc[:])
            nc.sync.dma_start(out=out[b * J : (b + 1) * J, :], in_=fo[:])

    nc.finalize()
    return nc


def prepare_in_maps(features, keypoint_coords, w_off1, b_off1, w_off2, b_off2,
                    w_att1, b_att1, w_att2, b_att2, n_cores=8):
    bf = ml_dtypes.bfloat16
    f32 = np.float32

    def w1t(w):  # [128, C] -> [128 k_local, Q, 128 m] bf16
        return np.ascontiguousarray(
            w.T.reshape(Q, 128, 128).transpose(1, 0, 2).astype(bf)
        )

    def wrap(flat):  # [N] int16 -> [128, N//16] gpsimd wrapped layout
        n = flat.shape[0]
        return np.tile(flat.reshape(n // 16, 16).T, (8, 1))

    w1o_h = w1t(np.asarray(w_off1, f32))
    w1a_h = w1t(np.asarray(w_att1, f32))
    w2o_h = np.ascontiguousarray(
        np.concatenate([w_off2[0::2], w_off2[1::2]], 0).T.astype(bf)
    )
    w2a_h = np.ascontiguousarray(np.asarray(w_att2, f32).T.astype(bf))
    b1o_h = np.asarray(b_off1, f32).reshape(128, 1).copy()
    b1a_h = np.asarray(b_att1, f32).reshape(128, 1).copy()
    b2o_h = np.concatenate([b_off2[0::2], b_off2[1::2]]).astype(f32).reshape(8, 1)
    b2a_h = np.asarray(b_att2, f32).reshape(4, 1).copy()
    posc_h = np.broadcast_to(np.arange(4, dtype=f32)[None, :], (128, 4)).copy()
    identb_h = np.eye(128, dtype=f32).astype(bf)

    # host-side keypoint geometry (all gather indices + seed weights derive
    # from keypoint_coords only)
    kp = np.asarray(keypoint_coords, f32)           # [32, J, 2]
    ix = (kp[..., 0] + 1.0) * 31.5                  # [32, J]
    iy = (kp[..., 1] + 1.0) * 31.5
    x0 = np.floor(ix); y0 = np.floor(iy)
    fx = ix - x0; fy = iy - y0
    bx = np.clip(np.round(ix) - 1.0, 0.0, 61.0)
    by = np.clip(np.round(iy) - 1.0, 0.0, 61.0)
    pos3 = np.arange(3, dtype=f32)
    rowidx = ((by[..., None] + pos3) * 64.0 + bx[..., None])  # [32, J, 3y]
    seedrow = ((y0[..., None] + pos3[:2]) * 64.0 + x0[..., None])  # [32,J,2y]

    in_maps = []
    for m in range(n_cores):
        bs = slice(B * m, B * (m + 1))
        feat_h = np.ascontiguousarray(
            np.asarray(features[bs], f32).transpose(0, 2, 3, 1).reshape(B * HW, C)
        ).astype(bf)
        idxg_h = np.empty((128, 64 + B * 24), np.int16)
        # seed idx: i = y*512 + b*J + j
        sflat = np.empty(2 * J * B, np.int16)
        for y in range(2):
            for b in range(B):
                sflat[y * J * B + b * J : y * J * B + (b + 1) * J] = (
                    seedrow[B * m + b, :, y] + b * HW
                ).astype(np.int16)
        idxg_h[:, 0:64] = wrap(sflat)
        # patch idx per b: i = y*J + j
        for b in range(B):
            flat = (rowidx[B * m + b].T.reshape(3 * J) + b * HW).astype(np.int16)
            idxg_h[:, 64 + b * 24 : 64 + (b + 1) * 24] = wrap(flat)
        # seed weights wsg[x, y, b*J+j] = wx(x)*wy(y)
        fxc = fx[bs].T  # [J, B]
        fyc = fy[bs].T
        wsg_h = np.empty((2, 2, J * B), f32)
        for x in range(2):
            for y in range(2):
                wx = (1.0 - fxc) if x == 0 else fxc
                wy = (1.0 - fyc) if y == 0 else fyc
                wsg_h[x, y] = (wx * wy).T.reshape(J * B)
        coef_h = np.ascontiguousarray(
            np.concatenate(
                [ix[bs].T, iy[bs].T, bx[bs].T, by[bs].T], axis=1
            ).astype(f32)
        )
        in_maps.append({
            "feat": feat_h, "idxg": idxg_h,
            "wsg": wsg_h.reshape(-1).astype(bf), "coef": coef_h,
            "w1o": w1o_h, "w1a": w1a_h, "w2o": w2o_h, "w2a": w2a_h,
            "b1o": b1o_h, "b1a": b1a_h, "b2o": b2o_h, "b2a": b2a_h,
            "posc": posc_h, "identb": identb_h,
        })
    return in_maps


_NC_CACHE = None


def get_nc():
    global _NC_CACHE
    if _NC_CACHE is None:
        _NC_CACHE = build_nc()
    return _NC_CACHE


def kernel(**inputs):
    from concourse.bass_utils import run_bass_kernel_spmd

    n_cores = 8
    nc = get_nc()
    in_maps = prepare_in_maps(**inputs, n_cores=n_cores)
    res = run_bass_kernel_spmd(
        nc, in_maps, core_ids=list(range(n_cores)),
        trace=bool(int(os.environ.get("KERNEL_TRACE", "0") or 0)),
    )
    kernel.last_results = res
    outs = [
        np.asarray(r["out"]).astype(np.float32).reshape(B, J, C)
        for r in res.results
    ]
    return np.concatenate(outs, axis=0)


# revision 9
# speedup vs baseline: 2.0613x; 1.0257x over previous
"""Trainium2 Bass kernel for nn_AdaptiveSampler (sparse grid_sample attention).

Strategy v3 (data-parallel over batch, 8 cores x 4 batch items each):
  - Host: features channels-last [B*H*W, C] bf16 (2KB rows). All gather
    indices depend only on keypoint_coords, so the host precomputes them
    in the gpsimd wrapped int16 layout, plus the per-column seed bilinear
    weights.
  - Device per core:
      seed: ONE transpose-gather of 2x2 corner cells as 2-cell rows
            (channel-major [128c, (x,q), (y,jb)]), then DVE multiply by
            per-column corner weights (partition-broadcast, 2x bf16 rate)
            and two halving adds -> seed lands directly channel-major,
            feeding the MLP matmuls with no PE transposes.
      patch: one 3x3-cell gather per batch item, keypoint-major
            ([128j, 3y, 3x*1024c]) for the fuse stage.
      MLPs: PE matmuls; offsets/attention transposed back per-b (PE).
      fuse: 9 diagonal-matmuls per batch item on PE: stationary
            diag(w9[:,xy]) built on the Scalar engine (activation with
            per-partition scale), moving = patch x-slices, accumulated in
            PSUM f32 -> output lands keypoint-major [j, C]; direct DMA out.
"""

import os
import sys
from contextlib import ExitStack

import numpy as np

sys.path.insert(0, "/opt/trn_rl_repo")

import ml_dtypes

import concourse.bass as bass
import concourse.tile as tile
from concourse import bacc, mybir

F32 = mybir.dt.float32
BF16 = mybir.dt.bfloat16
I16 = mybir.dt.int16

ALU = mybir.AluOpType
ACT = mybir.ActivationFunctionType
AX = mybir.AxisListType

B = 4          # batch items per core
C = 1024       # channels
H = W = 64
HW = H * W     # 4096 cells per batch item
J = 128        # keypoints
NP = 4         # sample points per keypoint
Q = C // 128   # 8 channel chunks
TWO23 = float(2 ** 23)


def _floor(nc, pool, src, shape, tag):
    """floor(src) on DVE via round-to-nearest + correction. Returns tile."""
    rnd = pool.tile(list(shape), F32, tag=f"floor_rnd_{tag}")
    nc.vector.tensor_scalar(rnd[:], src, TWO23, TWO23, ALU.add, ALU.subtract)
    flo = pool.tile(list(shape), F32, tag=f"floor_out_{tag}")
    nc.vector.tensor_tensor(flo[:], src, rnd[:], ALU.is_lt)
    nc.vector.tensor_tensor(flo[:], rnd[:], flo[:], ALU.subtract)
    return flo


def build_nc():
    nc = bacc.Bacc()

    feat = nc.declare_dram_parameter("feat", [B * HW, C], BF16, isOutput=False)
    # seed idx [128, 64] ++ per-b patch idx [128, 24] each
    idxg = nc.declare_dram_parameter("idxg", [128, 64 + B * 24], I16,
                                     isOutput=False)
    wsg = nc.declare_dram_parameter("wsg", [2 * 2 * J * B], BF16, isOutput=False)
    coef = nc.declare_dram_parameter("coef", [128, 16], F32, isOutput=False)
    w1o = nc.declare_dram_parameter("w1o", [128, Q, 128], BF16, isOutput=False)
    w1a = nc.declare_dram_parameter("w1a", [128, Q, 128], BF16, isOutput=False)
    w2o = nc.declare_dram_parameter("w2o", [128, 8], BF16, isOutput=False)
    w2a = nc.declare_dram_parameter("w2a", [128, 4], BF16, isOutput=False)
    b1o = nc.declare_dram_parameter("b1o", [128, 1], F32, isOutput=False)
    b1a = nc.declare_dram_parameter("b1a", [128, 1], F32, isOutput=False)
    b2o = nc.declare_dram_parameter("b2o", [8, 1], F32, isOutput=False)
    b2a = nc.declare_dram_parameter("b2a", [4, 1], F32, isOutput=False)
    posc = nc.declare_dram_parameter("posc", [128, 4], F32, isOutput=False)
    identb = nc.declare_dram_parameter("identb", [128, 128], BF16, isOutput=False)
    out = nc.declare_dram_parameter("out", [B * J, C], BF16, isOutput=True)

    # Overlapping row views of feat. 3-cell rows (patch): max start 16381.
    # 2-cell rows (seed): max start 16382. Read end == tensor end exactly.
    feat_ov3 = bass.AP(feat[:].tensor, 0, [[C, B * HW - 2], [1, 3 * C]])
    feat_ov2 = bass.AP(feat[:].tensor, 0, [[C, B * HW - 1], [1, 2 * C]])

    with ExitStack() as ctx:
        tc = ctx.enter_context(tile.TileContext(nc))
        cons = ctx.enter_context(tc.tile_pool(name="cons", bufs=1))
        gp = ctx.enter_context(tc.tile_pool(name="gpool", bufs=1))
        a = ctx.enter_context(tc.tile_pool(name="work", bufs=1))
        dgp = ctx.enter_context(tc.tile_pool(name="diag", bufs=2))
        ps = ctx.enter_context(tc.tile_pool(name="psT", bufs=2, space="PSUM"))
        pmm = ctx.enter_context(tc.tile_pool(name="psMM", bufs=2, space="PSUM"))
        pfu = ctx.enter_context(tc.tile_pool(name="psFU", bufs=3, space="PSUM"))

        # ---------------- constants ----------------
        def c_load(name, shape, dt, src):
            t = cons.tile(shape, dt, tag=name)
            nc.sync.dma_start(out=t[:], in_=src)
            return t

        # idx load FIRST so the gathers (critical path head) start ASAP;
        # remaining consts load while the gathers run.
        idxg_sb = c_load("idxg", [128, 64 + B * 24], I16, idxg[:])

        # ---------------- seed gather (channel-major, 4 chunks) -----------
        # chunk h: y = h//2, jb half = h%2 (256 idxs each)
        G2h = []
        for h in range(4):
            g2 = gp.tile([128, 16, 256], BF16, tag=f"G2{h}")
            nc.gpsimd.dma_gather(
                g2[:],
                feat_ov2,
                idxg_sb[:, 16 * h : 16 * h + 16],
                num_idxs=256,
                num_idxs_reg=256,
                elem_size=2 * C,
                elem_step=C,
                transpose=True,
            )
            G2h.append(g2)

        # ---------------- patch gathers (keypoint-major, one per b) -------
        Gt = []
        for b in range(B):
            g = gp.tile([128, 3, 3 * C], BF16, tag=f"G{b}")
            nc.gpsimd.dma_gather(
                g[:],
                feat_ov3,
                idxg_sb[:, 64 + b * 24 : 64 + (b + 1) * 24],
                num_idxs=3 * J,
                num_idxs_reg=3 * J,
                elem_size=3 * C,
                elem_step=C,
                transpose=False,
            )
            Gt.append(g)

        # ---------------- remaining constants (overlap the gathers) -------
        coef_sb = c_load("coef", [128, 16], F32, coef[:])
        w1o_sb = c_load("w1o", [128, Q, 128], BF16, w1o[:])
        w1a_sb = c_load("w1a", [128, Q, 128], BF16, w1a[:])
        w2o_sb = c_load("w2o", [128, 8], BF16, w2o[:])
        w2a_sb = c_load("w2a", [128, 4], BF16, w2a[:])
        b1o_sb = c_load("b1o", [128, 1], F32, b1o[:])
        b1a_sb = c_load("b1a", [128, 1], F32, b1a[:])
        b2o_sb = c_load("b2o", [8, 1], F32, b2o[:])
        b2a_sb = c_load("b2a", [4, 1], F32, b2a[:])
        posc_sb = c_load("posc", [128, 4], F32, posc[:])
        idb_sb = c_load("identb", [128, 128], BF16, identb[:])
        wsg_sb = cons.tile([128, 2, 2, J * B], BF16, tag="wsg")
        nc.sync.dma_start(
            out=wsg_sb[:],
            in_=wsg[:].unsqueeze(0).to_broadcast((128, 2 * 2 * J * B)),
        )

        ixv = coef_sb[:, 0:4]    # [J, B] pixel x coords
        iyv = coef_sb[:, 4:8]
        bxv = coef_sb[:, 8:12]   # patch x base (f32 integer-valued)
        byv = coef_sb[:, 12:16]

        # ---------------- seed combine (DVE, 2x-rate contiguous ops) ------
        seed = a.tile([128, Q, J * B], BF16)
        with nc.allow_low_precision("bf16 grid-sample compute"):
            for h in range(4):
                y, jh = h // 2, h % 2
                g2 = G2h[h]
                # [128, x2, q8, 256jb] *= wsg[x, y(h), jb-slice]
                g2v = g2[:].rearrange("p (x q) i -> p x q i", x=2)
                nc.vector.tensor_tensor(
                    g2v,
                    g2v,
                    wsg_sb[:, :, y, 256 * jh : 256 * jh + 256]
                    .unsqueeze(2)
                    .to_broadcast((128, 2, Q, 256)),
                    ALU.mult,
                )
                # x-add: first half += second half (contiguous)
                g2f = g2[:].rearrange("p e i -> p (e i)")
                nc.vector.tensor_tensor(
                    g2f[0:128, 0 : Q * 256],
                    g2f[0:128, 0 : Q * 256],
                    g2f[0:128, Q * 256 : 2 * Q * 256],
                    ALU.add,
                )
            # y-add into contiguous seed tile, per jb half
            for jh in range(2):
                nc.vector.tensor_tensor(
                    seed[:, :, 256 * jh : 256 * jh + 256],
                    G2h[jh][:, 0:Q, :],
                    G2h[2 + jh][:, 0:Q, :],
                    ALU.add,
                )

        # ---------------- MLPs (PE) ----------------
        def mlp_head(w1_sb, b1_sb, name):
            hps = pmm.tile([128, J * B], F32, tag="mlp")
            for q in range(Q):
                nc.tensor.matmul(
                    hps[:],
                    w1_sb[:, q, :],
                    seed[:, q, :],
                    start=(q == 0),
                    stop=(q == Q - 1),
                )
            h_sb = a.tile([128, J * B], BF16, tag=f"hsb_{name}")
            nc.scalar.activation(h_sb[:], hps[:], ACT.Relu, bias=b1_sb[:])
            return h_sb

        h_off = mlp_head(w1o_sb, b1o_sb, "off")
        h_att = mlp_head(w1a_sb, b1a_sb, "att")

        ops2 = pmm.tile([8, J * B], F32, tag="mlp")
        nc.tensor.matmul(ops2[:], w2o_sb[:], h_off[:], start=True, stop=True)
        off2 = a.tile([8, J * B], BF16)
        nc.scalar.activation(off2[:], ops2[:], ACT.Identity, bias=b2o_sb[:])

        aps2 = pmm.tile([4, J * B], F32, tag="mlp")
        nc.tensor.matmul(aps2[:], w2a_sb[:], h_att[:], start=True, stop=True)
        att2 = a.tile([4, J * B], BF16)
        nc.scalar.activation(att2[:], aps2[:], ACT.Identity, bias=b2a_sb[:])

        # transpose MLP outputs back to [J, B, ch] (contiguous b-blocks)
        offT = a.tile([J, B, 8], F32)
        attT = a.tile([J, B, 4], F32)
        for b in range(B):
            pso = ps.tile([128, 8], BF16, tag="tp")
            nc.tensor.transpose(
                pso[:, 0:8], off2[:, b * J : (b + 1) * J], idb_sb[:8, :8]
            )
            nc.scalar.copy(offT[:, b, :], pso[:, 0:8])
            psa = ps.tile([128, 4], BF16, tag="tp")
            nc.tensor.transpose(
                psa[:, 0:4], att2[:, b * J : (b + 1) * J], idb_sb[:4, :4]
            )
            nc.scalar.copy(attT[:, b, :], psa[:, 0:4])

        # ---------------- fuse weights (DVE, batched [J, B, ...]) ---------
        px = a.tile([J, B, NP], F32)
        nc.vector.tensor_tensor(
            px[:],
            ixv.unsqueeze(2).to_broadcast((J, B, NP)),
            offT[:, :, 0:NP],
            ALU.add,
        )
        py = a.tile([J, B, NP], F32)
        nc.vector.tensor_tensor(
            py[:],
            iyv.unsqueeze(2).to_broadcast((J, B, NP)),
            offT[:, :, NP : 2 * NP],
            ALU.add,
        )

        # softmax over NP  [J, B, NP]
        amax = a.tile([J, B, 1], F32)
        nc.vector.tensor_reduce(amax[:], attT[:], AX.X, ALU.max)
        ae = a.tile([J, B, NP], F32)
        nc.vector.tensor_tensor(
            ae[:], attT[:], amax[:].to_broadcast((J, B, NP)), ALU.subtract
        )
        nc.scalar.activation(ae[:], ae[:], ACT.Exp)
        asum = a.tile([J, B, 1], F32)
        nc.vector.tensor_reduce(asum[:], ae[:], AX.X, ALU.add)
        nc.vector.reciprocal(asum[:], asum[:])
        attw = a.tile([J, B, NP], F32)
        nc.vector.tensor_tensor(
            attw[:], ae[:], asum[:].to_broadcast((J, B, NP)), ALU.mult
        )

        def axis_select(pc, base, tagp):
            """Position-select weights [J, B, NP, 3]:
            w0*(pos==d) + w1*(pos==d+1), d = floor(pc) - base."""
            c0 = _floor(nc, a, pc[:], (J, B, NP), tagp)
            w1t = a.tile([J, B, NP], F32, tag=f"{tagp}_w1")
            nc.vector.tensor_tensor(w1t[:], pc[:], c0[:], ALU.subtract)
            w0t = a.tile([J, B, NP], F32, tag=f"{tagp}_w0")
            nc.vector.tensor_scalar(w0t[:], w1t[:], -1.0, 1.0, ALU.mult, ALU.add)
            d = a.tile([J, B, NP], F32, tag=f"{tagp}_d")
            nc.vector.tensor_tensor(
                d[:], c0[:], base.unsqueeze(2).to_broadcast((J, B, NP)),
                ALU.subtract,
            )
            d1 = a.tile([J, B, NP], F32, tag=f"{tagp}_d1")
            nc.vector.tensor_scalar_add(d1[:], d[:], 1.0)
            posb = (
                posc_sb[:, 0:3]
                .unsqueeze(1)
                .unsqueeze(2)
                .to_broadcast((J, B, NP, 3))
            )
            sel = a.tile([J, B, NP, 3], F32, tag=f"{tagp}_sel")
            eq = a.tile([J, B, NP, 3], F32, tag=f"{tagp}_eq")
            nc.vector.tensor_tensor(
                eq[:], d[:].unsqueeze(3).to_broadcast((J, B, NP, 3)), posb,
                ALU.is_equal,
            )
            nc.vector.tensor_tensor(
                sel[:], eq[:], w0t[:].unsqueeze(3).to_broadcast((J, B, NP, 3)),
                ALU.mult,
            )
            nc.vector.tensor_tensor(
                eq[:], d1[:].unsqueeze(3).to_broadcast((J, B, NP, 3)), posb,
                ALU.is_equal,
            )
            nc.vector.tensor_tensor(
                eq[:], eq[:], w1t[:].unsqueeze(3).to_broadcast((J, B, NP, 3)),
                ALU.mult,
            )
            nc.vector.tensor_tensor(sel[:], sel[:], eq[:], ALU.add)
            return sel

        wxsel = axis_select(px, bxv, "sx")
        wysel = axis_select(py, byv, "sy")

        # fold attention weight into y-selects: ty [J, B, NP, 3Y]
        ty = a.tile([J, B, NP, 3], F32)
        nc.vector.tensor_tensor(
            ty[:], wysel[:], attw[:].unsqueeze(3).to_broadcast((J, B, NP, 3)),
            ALU.mult,
        )
        # per-cell weights w9 [J, B, 3Y, 3X] = sum_n ty[n, Y] * wxsel[n, X]
        w9 = a.tile([J, B, 3, 3], F32)
        tmp9 = a.tile([J, B, 3, 3], F32)
        for n in range(NP):
            dst = (w9 if n == 0 else tmp9)
            nc.vector.tensor_tensor(
                dst[:],
                ty[:, :, n, :].unsqueeze(3).to_broadcast((J, B, 3, 3)),
                wxsel[:, :, n, :].unsqueeze(2).to_broadcast((J, B, 3, 3)),
                ALU.mult,
            )
            if n > 0:
                nc.vector.tensor_tensor(w9[:], w9[:], tmp9[:], ALU.add)

        # ---------------- fuse (PE diagonal matmuls, per batch item) ------
        w9b = a.tile([J, B, 9], BF16)
        nc.vector.tensor_copy(w9b[:], w9[:].rearrange("j b y x -> j b (y x)"))
        for b in range(B):
            dgs = []
            for k in range(9):
                dg = dgp.tile([128, 128], BF16, tag=f"dg{k}")
                nc.vector.tensor_tensor(
                    dg[:],
                    idb_sb[:],
                    w9b[:, b, k : k + 1].to_broadcast((128, 128)),
                    ALU.mult,
                )
                dgs.append(dg)
            fo = a.tile([128, C], BF16, tag=f"fo{b}")
            for hh in range(2):
                acc = pfu.tile([128, 512], F32, tag="facc")
                k = 0
                for y in range(3):
                    for x in range(3):
                        nc.tensor.matmul(
                            acc[:],
                            dgs[k][:],
                            Gt[b][:, y, x * C + hh * 512 : x * C + hh * 512 + 512],
                            start=(k == 0),
                            stop=(k == 8),
                        )
                        k += 1
                nc.scalar.copy(fo[:, hh * 512 : (hh + 1) * 512], acc[:])
            nc.sync.dma_start(out=out[b * J : (b + 1) * J, :], in_=fo[:])

    nc.finalize()
    return nc


def prepare_in_maps(features, keypoint_coords, w_off1, b_off1, w_off2, b_off2,
                    w_att1, b_att1, w_att2, b_att2, n_cores=8):
    bf = ml_dtypes.bfloat16
    f32 = np.float32

    def w1t(w):  # [128, C] -> [128 k_local, Q, 128 m] bf16
        return np.ascontiguousarray(
            w.T.reshape(Q, 128, 128).transpose(1, 0, 2).astype(bf)
        )

    def wrap(flat):  # [N] int16 -> [128, N//16] gpsimd wrapped layout
        n = flat.shape[0]
        return np.tile(flat.reshape(n // 16, 16).T, (8, 1))

    w1o_h = w1t(np.asarray(w_off1, f32))
    w1a_h = w1t(np.asarray(w_att1, f32))
    w2o_h = np.ascontiguousarray(
        np.concatenate([w_off2[0::2], w_off2[1::2]], 0).T.astype(bf)
    )
    w2a_h = np.ascontiguousarray(np.asarray(w_att2, f32).T.astype(bf))
    b1o_h = np.asarray(b_off1, f32).reshape(128, 1).copy()
    b1a_h = np.asarray(b_att1, f32).reshape(128, 1).copy()
    b2o_h = np.concatenate([b_off2[0::2], b_off2[1::2]]).astype(f32).reshape(8, 1)
    b2a_h = np.asarray(b_att2, f32).reshape(4, 1).copy()
    posc_h = np.broadcast_to(np.arange(4, dtype=f32)[None, :], (128, 4)).copy()
    identb_h = np.eye(128, dtype=f32).astype(bf)

    # host-side keypoint geometry (all gather indices + seed weights derive
    # from keypoint_coords only)
    kp = np.asarray(keypoint_coords, f32)           # [32, J, 2]
    ix = (kp[..., 0] + 1.0) * 31.5                  # [32, J]
    iy = (kp[..., 1] + 1.0) * 31.5
    x0 = np.floor(ix); y0 = np.floor(iy)
    fx = ix - x0; fy = iy - y0
    bx = np.clip(np.round(ix) - 1.0, 0.0, 61.0)
    by = np.clip(np.round(iy) - 1.0, 0.0, 61.0)
    pos3 = np.arange(3, dtype=f32)
    rowidx = ((by[..., None] + pos3) * 64.0 + bx[..., None])  # [32, J, 3y]
    seedrow = ((y0[..., None] + pos3[:2]) * 64.0 + x0[..., None])  # [32,J,2y]

    in_maps = []
    for m in range(n_cores):
        bs = slice(B * m, B * (m + 1))
        feat_h = np.ascontiguousarray(
            np.asarray(features[bs], f32).transpose(0, 2, 3, 1).reshape(B * HW, C)
        ).astype(bf)
        idxg_h = np.empty((128, 64 + B * 24), np.int16)
        # seed idx: i = y*512 + b*J + j
        sflat = np.empty(2 * J * B, np.int16)
        for y in range(2):
            for b in range(B):
                sflat[y * J * B + b * J : y * J * B + (b + 1) * J] = (
                    seedrow[B * m + b, :, y] + b * HW
                ).astype(np.int16)
        idxg_h[:, 0:64] = wrap(sflat)
        # patch idx per b: i = y*J + j
        for b in range(B):
            flat = (rowidx[B * m + b].T.reshape(3 * J) + b * HW).astype(np.int16)
            idxg_h[:, 64 + b * 24 : 64 + (b + 1) * 24] = wrap(flat)
        # seed weights wsg[x, y, b*J+j] = wx(x)*wy(y)
        fxc = fx[bs].T  # [J, B]
        fyc = fy[bs].T
        wsg_h = np.empty((2, 2, J * B), f32)
        for x in range(2):
            for y in range(2):
                wx = (1.0 - fxc) if x == 0 else fxc
                wy = (1.0 - fyc) if y == 0 else fyc
                wsg_h[x, y] = (wx * wy).T.reshape(J * B)
        coef_h = np.ascontiguousarray(
            np.concatenate(
                [ix[bs].T, iy[bs].T, bx[bs].T, by[bs].T], axis=1
            ).astype(f32)
        )
        in_maps.append({
            "feat": feat_h, "idxg": idxg_h,
            "wsg": wsg_h.reshape(-1).astype(bf), "coef": coef_h,
            "w1o": w1o_h, "w1a": w1a_h, "w2o": w2o_h, "w2a": w2a_h,
            "b1o": b1o_h, "b1a": b1a_h, "b2o": b2o_h, "b2a": b2a_h,
            "posc": posc_h, "identb": identb_h,
        })
    return in_maps


_NC_CACHE = None


def get_nc():
    global _NC_CACHE
    if _NC_CACHE is None:
        _NC_CACHE = build_nc()
    return _NC_CACHE


def kernel(**inputs):
    from concourse.bass_utils import run_bass_kernel_spmd

    n_cores = 8
    nc = get_nc()
    in_maps = prepare_in_maps(**inputs, n_cores=n_cores)
    res = run_bass_kernel_spmd(
        nc, in_maps, core_ids=list(range(n_cores)),
        trace=bool(int(os.environ.get("KERNEL_TRACE", "0") or 0)),
    )
    kernel.last_results = res
    outs = [
        np.asarray(r["out"]).astype(np.float32).reshape(B, J, C)
        for r in res.results
    ]
    return np.concatenate(outs, axis=0)
